# revision 45
# baseline (speedup 1.0000x reference)
"""Trainium2 Bass kernel for AngularLocalSensitiveHashing (8-core data parallel).

kernel(x, random_matrix) -> (x_sorted, sorted_hashes, sorted_indices)

Sharding: core b handles batch b for all 8 rounds (hashing, counting sort,
gathers are independent per batch; random matrices replicated).

Per-core device algorithm (all engines, see inline comments):
  1. rot = x @ rm^T per round (PE, fp32), m = max_c |rot| (VE abs-max reduce)
  2. d[k, pos] = [rot | -rot]^T - m  (PE, psum accumulate); one-hot
     OT = (d >= 0) (VE tensor_scalar, exact since both halves of the
     comparison come from bit-identical products)
  3. per-bucket running counts G = inclusive prefix scan of OT along
     positions (VE tensor_tensor_scan), histogram = last column,
     CH = exclusive bucket prefix (PE matmul with strict-upper ones)
  4. dest = CH[h] + rank extracted via matmuls against the one-hot:
     rows = [CH_hi | CH_lo | bucket-id] @ OT + [ones|0|0] @ (OT * G)
  5. PE-transpose extraction rows to per-position columns, cast to int32
  6. one indirect DMA scatter per round writes the 512 B x rows to their
     sorted positions; a second scatter writes (index, hash) pairs.
"""
from contextlib import ExitStack

import numpy as np

import concourse.mybir as mybir
import concourse.tile as tile
from concourse import bacc
from concourse import bass_utils
from concourse.bass import IndirectOffsetOnAxis

F32 = mybir.dt.float32
F16 = mybir.dt.float16
I32 = mybir.dt.int32

R, L, D = 8, 4096, 128
NBLK, BLK = 8, 512      # position blocks per round
NCH = 32                # chunks of 128 positions
B = 8                   # batches = cores


def host_inputs(x_b: np.ndarray, rm: np.ndarray) -> dict[str, np.ndarray]:
    """Per-core input dict from x[b] (4096, 128) f32 and rm (16, 64, 128) f32."""
    xt = np.ascontiguousarray(x_b.T)
    xr = np.ascontiguousarray(x_b.reshape(NCH, 128, D).transpose(1, 0, 2))
    rmt = np.ascontiguousarray(rm[:R].transpose(2, 0, 1).reshape(D, R * 64))
    ust = np.triu(np.ones((128, 128), np.float16), k=1)
    lhsp = np.zeros((128, 3), np.float16)
    lhsp[:, 0] = 1.0
    wcol = np.arange(1, 129, dtype=np.float16).reshape(128, 1)
    ident = np.eye(128, dtype=np.float32)
    sel4 = np.zeros((4, 4 * 128), np.float32)
    for rr in range(4):
        sel4[rr, rr * 128:(rr + 1) * 128] = 1.0
    nident = -np.eye(128, dtype=np.float32)
    iotaf = (np.arange(NCH)[None, :] * 128
             + np.arange(128)[:, None]).astype(np.float32)
    return {
        "xt": xt.astype(np.float32),
        "xr": xr.astype(np.float32),
        "rmt": rmt.astype(np.float32),
        "ust": ust,
        "lhsp": lhsp,
        "wcol": wcol,
        "ident": ident,
        "nident": nident,
        "sel4": sel4,
        "iotaf": iotaf,
    }


INPUT_SPECS = {
    "xt": ((D, L), F32),
    "xr": ((128, NCH, D), F32),
    "rmt": ((D, R * 64), F32),
    "ust": ((128, 128), F16),
    "lhsp": ((128, 3), F16),
    "wcol": ((128, 1), F16),
    "ident": ((128, 128), F32),
    "nident": ((128, 128), F32),
    "sel4": ((4, 4 * 128), F32),
    "iotaf": ((128, NCH), F32),
}

PAD = 64  # padded pair row (256 B minimum scatter element)

OUTPUT_SPECS = {
    "xs": ((R * L, D), F32),
    "pairpad": ((R * L, PAD), F32),
}


def lsh_body(ctx: ExitStack, tc: tile.TileContext, ins: dict, outs: dict):
    nc = tc.nc
    A = mybir.AluOpType

    const = ctx.enter_context(tc.tile_pool(name="const", bufs=1))
    work = ctx.enter_context(tc.tile_pool(name="work", bufs=2))
    psum = ctx.enter_context(tc.tile_pool(name="psum", bufs=2, space="PSUM"))
    psum_sm = ctx.enter_context(tc.tile_pool(name="psum_sm", bufs=2, space="PSUM"))

    # ---- persistent inputs ----
    xt = const.tile([D, L], F32)
    nc.sync.dma_start(xt[:], ins["xt"][:])
    xr = const.tile([128, NCH, D], F32)
    nc.sync.dma_start(xr[:], ins["xr"][:])
    rmt = const.tile([D, R * 64], F32)
    nc.sync.dma_start(rmt[:], ins["rmt"][:])
    ust = const.tile([128, 128], F16)
    nc.sync.dma_start(ust[:], ins["ust"][:])
    lhsp = const.tile([128, 3], F16)
    nc.sync.dma_start(lhsp[:], ins["lhsp"][:])
    wcol = const.tile([128, 1], F16)
    nc.sync.dma_start(wcol[:], ins["wcol"][:])
    ident = const.tile([128, 128], F32)
    nc.sync.dma_start(ident[:], ins["ident"][:])
    nident = const.tile([128, 128], F32)
    nc.sync.dma_start(nident[:], ins["nident"][:])
    sel4 = const.tile([4, 4 * 128], F32)
    nc.sync.dma_start(sel4[:], ins["sel4"][:])
    zcol16 = const.tile([128, 1], F16)
    nc.vector.memset(zcol16[:], 0.0)
    # pair payload: [:, :, 0] = original position, [:, :, 1] = hash (per round)
    iotaf = const.tile([128, NCH], F32)
    nc.sync.dma_start(iotaf[:], ins["iotaf"][:])
    pairs64 = const.tile([128, NCH, PAD], F32)
    nc.vector.memset(pairs64[:], 0.0)
    nc.vector.tensor_copy(pairs64[:, :, 0], iotaf[:])

    mall = const.tile([128, NCH, R], F32)      # per-position abs-max
    negmrow = const.tile([4, L], F32)          # -m rows for current half
    # +rot staged for bit-exact PE transposes into bucket-major (one half)
    rotp = const.tile([128, NCH, 4, 64], F32)
    # extraction rows: round rr -> partitions 32*rr..+2
    rowbuf = const.tile([128, L], F32)
    destall = const.tile([128, NCH, 12], F32)

    for half in range(2):
        # ---- stage 1: rot chunks + abs-max reduce + staging (4 rounds) ----
        for c in range(NCH):
            rot_ps = psum.tile([128, 256], F32, tag="d")
            nc.tensor.matmul(rot_ps[:], lhsT=xt[:, c * 128:(c + 1) * 128],
                             rhs=rmt[:, half * 256:(half + 1) * 256],
                             start=True, stop=True)
            rot3 = rot_ps[:].rearrange("p (r c) -> p r c", r=4)
            nc.vector.tensor_reduce(
                out=mall[:, c, half * 4:(half + 1) * 4], in_=rot3,
                axis=mybir.AxisListType.X, op=A.max, apply_absolute_value=True)
            nc.scalar.copy(rotp[:, c, :, :], rot3)

        # ---- stage 2: m columns -> -m rows ----
        for c in range(NCH):
            mt_ps = psum_sm.tile([4, 128], F32, tag="sm")
            nc.tensor.transpose(mt_ps[:], mall[:, c, half * 4:(half + 1) * 4],
                                ident[:])
            nc.scalar.activation(negmrow[:, c * 128:(c + 1) * 128], mt_ps[:],
                                 mybir.ActivationFunctionType.Copy, scale=-1.0)

        # ---- stage 3: per-round pipeline ----
        # d[k, pos] = [+rot|-rot]^T - m built by bit-exact PE transposes
        # (rhs = +/- identity) plus a -m broadcast accumulate.
        for rr in range(4):
            r = half * 4 + rr
            OT = work.tile([128, L], F16, tag="OT")
            for blk in range(NBLK):
                sl = slice(blk * BLK, (blk + 1) * BLK)
                d_ps = psum.tile([128, BLK], F32, tag="d")
                nc.tensor.matmul(d_ps[:], lhsT=sel4[:, rr * 128:(rr + 1) * 128],
                                 rhs=negmrow[:, sl], start=True, stop=True)
                for j in range(4):
                    c = blk * 4 + j
                    nc.tensor.matmul(d_ps[0:64, j * 128:(j + 1) * 128],
                                     lhsT=rotp[:, c, rr, :], rhs=ident[:],
                                     start=False, stop=False,
                                     skip_group_check=True)
                    nc.tensor.matmul(d_ps[64:128, j * 128:(j + 1) * 128],
                                     lhsT=rotp[:, c, rr, :], rhs=nident[:],
                                     start=False, stop=False,
                                     skip_group_check=True)
                nc.vector.tensor_scalar(out=OT[:, sl], in0=d_ps[:],
                                        scalar1=0.0, scalar2=None, op0=A.is_ge)

            G = work.tile([128, L], F16, tag="G")
            nc.vector.tensor_tensor_scan(G[:], data0=OT[:],
                                         data1=zcol16[:].to_broadcast([128, L]),
                                         initial=0.0, op0=A.add, op1=A.add)

            P = work.tile([128, L], F16, tag="P")
            nc.vector.tensor_mul(P[:], OT[:], G[:])

            # CH' = exclusive bucket prefix + (4096 r - 1), hi/lo split, fp16
            ch_ps = psum_sm.tile([128, 1], F32, tag="sm")
            nc.tensor.matmul(ch_ps[:], lhsT=ust[:], rhs=G[:, L - 1:L],
                             start=True, stop=True)
            chp = work.tile([128, 1], F32, tag="chp")
            nc.vector.tensor_scalar_add(chp[:], ch_ps[:], float(L * r - 1))
            chp_i = work.tile([128, 2], I32, tag="chp_i")
            nc.vector.tensor_copy(chp_i[:, 0:1], chp[:])
            nc.vector.tensor_single_scalar(chp_i[:, 1:2], chp_i[:, 0:1], 255,
                                           A.bitwise_and)
            nc.vector.tensor_single_scalar(chp_i[:, 0:1], chp_i[:, 0:1], -256,
                                           A.bitwise_and)
            lhse = work.tile([128, 3], F16, tag="lhse")
            nc.vector.tensor_copy(lhse[:, 0:2], chp_i[:])
            nc.vector.tensor_copy(lhse[:, 2:3], wcol[:])

            # extraction: row0 = CHhi@OT + ones@P; row1 = CHlo@OT; row2 = w@OT
            for blk in range(NBLK):
                sl = slice(blk * BLK, (blk + 1) * BLK)
                ext_ps = psum.tile([3, BLK], F32, tag="ext")
                nc.tensor.matmul(ext_ps[:], lhsT=lhse[:], rhs=OT[:, sl],
                                 start=True, stop=False)
                nc.tensor.matmul(ext_ps[:], lhsT=lhsp[:], rhs=P[:, sl],
                                 start=False, stop=True)
                nc.scalar.copy(rowbuf[32 * rr:32 * rr + 3, sl], ext_ps[:])

        # ---- stage 4: transpose extraction rows to per-position columns ----
        for c in range(NCH):
            tr_ps = psum_sm.tile([128, 128], F32, tag="tr")
            nc.tensor.transpose(tr_ps[:], rowbuf[:, c * 128:(c + 1) * 128],
                                ident[:])
            nc.scalar.copy(
                destall[:, c, :].rearrange("p (q f) -> p q f", q=4),
                tr_ps[:].rearrange("p (q f) -> p q f", q=4)[:, :, 0:3])

        # ---- stage 5: wrapped int16 indices + scatters per round ----
        wrapd = ins["wrapbuf"]  # (R, L) int16 internal DRAM scratch
        for rr in range(4):
            r = half * 4 + rr
            q3 = 3 * rr
            offs_f = work.tile([128, NCH], F32, tag="offs_f")
            nc.vector.tensor_add(offs_f[:], destall[:, :, q3],
                                 destall[:, :, q3 + 1])
            o16 = work.tile([128, NCH], mybir.dt.int16, tag="o16")
            nc.vector.tensor_copy(o16[:], offs_f[:])
            # wrap[q*256 + c*8 + a] = o16[16a+q, c]; iterate (a, q, c)
            nc.sync.dma_start(
                out=wrapd[r].rearrange("(q c a) -> a q c", q=16, c=NCH, a=8),
                in_=o16[:])
            idxs = work.tile([128, L // 16], mybir.dt.int16, tag="idxs")
            for g in range(8):
                nc.sync.dma_start(idxs[16 * g:16 * (g + 1), :],
                                  wrapd[r].rearrange("(q s) -> q s", q=16))

            nc.vector.tensor_copy(pairs64[:, :, 1], destall[:, :, q3 + 2])
            nc.gpsimd.dma_scatter_add(outs["xs"][:], xr[:], idxs[:], L, L, D)
            nc.gpsimd.dma_scatter_add(outs["pairpad"][:], pairs64[:], idxs[:],
                                      L, L, PAD)


_BUILD_CACHE: dict = {}


def build_nc():
    if "nc" in _BUILD_CACHE:
        return _BUILD_CACHE["nc"]
    nc = bacc.Bacc("TRN2", target_bir_lowering=False, debug=False,
                   num_devices=B)
    ins = {name: nc.dram_tensor(name, list(shape), dt, kind="ExternalInput").ap()
           for name, (shape, dt) in INPUT_SPECS.items()}
    ins["wrapbuf"] = nc.dram_tensor("wrapbuf", [R, L], mybir.dt.int16,
                                    kind="Internal").ap()
    outs = {name: nc.dram_tensor(name, list(shape), dt, kind="ExternalOutput").ap()
            for name, (shape, dt) in OUTPUT_SPECS.items()}
    with tile.TileContext(nc) as tc:
        with ExitStack() as ctx:
            lsh_body(ctx, tc, ins, outs)
    nc.compile()
    _BUILD_CACHE["nc"] = nc
    return nc


def run(x, random_matrix, trace=False, **kwargs):
    nc = build_nc()
    x = np.asarray(x, dtype=np.float32)
    rm = np.asarray(random_matrix, dtype=np.float32)
    in_maps = [host_inputs(x[b], rm) for b in range(B)]
    res = bass_utils.run_bass_kernel_spmd(
        nc, in_maps, core_ids=list(range(B)), trace=trace, **kwargs)

    x_sorted = np.empty((R, B, L // 64, 64, D), np.float32)
    sorted_hashes = np.empty((R, B, L // 64, 64), np.int32)
    sorted_indices = np.empty((R, B, L, 1), np.int32)
    for b in range(B):
        m = res.results[b]["merged"]
        x_sorted[:, b] = m[:, 0:D].reshape(R, L // 64, 64, D)
        pair = m[:, D:D + 2].astype(np.int32).reshape(R, L, 2)
        sorted_indices[:, b] = pair[:, :, 0:1]
        sorted_hashes[:, b] = pair[:, :, 1].reshape(R, L // 64, 64)
    return (x_sorted, sorted_hashes, sorted_indices), res


def kernel(x: np.ndarray, random_matrix: np.ndarray):
    outs, _ = run(x, random_matrix, trace=False)
    return outs
